# revision 30
# baseline (speedup 1.0000x reference)
"""Trainium2 Bass kernel for nn_CADenseAdd (context-adaptive low-rank dense + ReLU).

Reference math (per batch row b):
    s_b   = S + context_b @ W                  # [RANK]
    out_b = relu((x_b @ U) * s_b @ V.T + bias) # [UNITS]

Sharding: data-parallel over batch B=2048 across 8 cores (256 rows/core);
U/S/V/W replicated.  All matmuls are done "transposed" so the contraction
dim always lands on SBUF partitions with zero on-device transposes:

    sT  = W^T @ ctxT + S          [RANK,  BS]
    xuT = U^T @ xT                [RANK,  BS]
    tT  = xuT * sT  (cast fp16)   [RANK,  BS]
    outT[um] = Vt[um] @ tT        [UNITS, BS]  (bias + ReLU fused on ScalarE)

The host packs every operand into [128, ...] partition-major contiguous
layout (part of input sharding) and casts matmul operands to fp16; PSUM
accumulation is fp32, the elementwise xu*s is fp32, so the end-to-end
relative error stays ~1e-3 vs the fp32 reference.
"""

import re

import numpy as np

import bass_rust
import concourse.bass as bass
import concourse.tile as tile
from concourse import mybir
from concourse.bass_utils import run_bass_kernel_spmd
from concourse.vector_clock import ScopedClock


def _split_drain_and_barrier(self, tick_clock, wait_clock):
    """Replacement for TileContext._drain_and_barrier.

    The walrus build in this toolchain cannot encode more than one sync
    wait per instruction ("Too many sync wait commands"), and Tile's final
    drain carries one wait per active proc (~12 here).  Emit those waits as
    a chain of single-wait SP nops instead, then a bare drain: the SP queue
    executes in order, so the drain still happens after every proc's final
    tick.
    """
    ticks = [int(x) for x in re.findall(r"\d+", repr(tick_clock.global_clock))]
    for proc, tick in enumerate(ticks):
        if tick > 0:
            nop_inst = self.nc.sync.nop(nofuse=True)
            sub = bass_rust.VectorClock()
            sub.require_at_least(proc, tick)
            wait_clock.add_sem_waits(nop_inst.ins, ScopedClock({None: sub}))
    self.nc.sync.drain()
    self.nc.all_engine_barrier()
    popped = self.nc._tile_sem_poison_stack.pop()
    assert popped is self._sem_poison
    self.nc.clear_and_free_semaphores(list(self.sems.allocated().values()))
    self.nc.all_engine_barrier()


tile.TileContext._drain_and_barrier = _split_drain_and_barrier

# Problem shape (hardcoded per contract)
M = 8  # cores
B, N, C = 2048, 4096, 1024
UNITS, RANK = 4096, 512
BS = B // M  # 256 rows per core
P = 128
KN = N // P      # 32 contraction tiles for x @ U
KC = C // P      # 8 contraction tiles for ctx @ W
RM = RANK // P   # 4 tiles of RANK
UM = UNITS // P  # 32 tiles of UNITS

F16 = mybir.dt.float16
F32 = mybir.dt.float32

# DMA chunking (chosen so the sync-ring FIFO delivers data in need-order
# with ~0.5 MB granularity: compute starts after ~1.5 MB of input).
X_CHUNK = 8   # kn tiles per x DMA      (4 DMAs x 512 KB)
U_CHUNK = 4   # kn tiles per U DMA      (8 DMAs x 512 KB)
W_CHUNK = 4   # kc tiles per W/ctx DMA  (2 DMAs each)
V_CHUNK = 4   # um tiles per V DMA      (8 DMAs x 512 KB)
O_CHUNK = 8   # um tiles per out store group (4 groups, 2 stores each)


N_WARM_MM = 16  # ~3.4us of cold-rate matmuls: spans the HAM SHORT window


def build_program(zero_bias: bool = True) -> bass.Bass:
    """Build the per-core SPMD program.

    Wait-encoding constraint: this walrus build cannot encode >1 sem-wait
    on DVE/ACT tensor instructions (setupSyncWait "Too many sync wait
    commands"), while matmuls were observed to encode 2.  The structure
    below keeps every DVE/ACT instruction at <=1 wait: each engine
    "pre-touches" its DMA-sourced operands once (so later instructions
    only wait on PE), PSUM banks are never shared across phases
    (2+4+2 = 8 banks), and output staging tiles are never reused.
    """
    nc = bass.Bass("TRN2", debug=False)

    # S is folded into mm1 on the host: ctxT/W carry an extra contraction
    # tile (ones-row / S-row), so sT = W_aug^T @ ctxT_aug exactly.
    KC1 = KC + 1
    xT_d = nc.dram_tensor("xT", [P, KN, BS], F16, kind="ExternalInput").ap()
    ctxT_d = nc.dram_tensor("ctxT", [P, KC1, BS], F16, kind="ExternalInput").ap()
    U_d = nc.dram_tensor("U", [P, KN, RANK], F16, kind="ExternalInput").ap()
    W_d = nc.dram_tensor("W", [P, KC1, RANK], F16, kind="ExternalInput").ap()
    V3_d = nc.dram_tensor("V3", [P, UM, RM, P], F16, kind="ExternalInput").ap()
    if not zero_bias:
        bias_d = nc.dram_tensor("bias", [P, UM], F32, kind="ExternalInput").ap()
    outT_d = nc.dram_tensor("outT", [P, UM, BS], F16, kind="ExternalOutput").ap()

    with tile.TileContext(nc) as tc:
        with (
            tc.tile_pool(name="consts", bufs=1) as cpool,
            tc.tile_pool(name="ctxp", bufs=1) as ctxpool,
            tc.tile_pool(name="wp", bufs=1) as wpool,
            tc.tile_pool(name="xp", bufs=1) as xpool,
            tc.tile_pool(name="up", bufs=1) as upool,
            tc.tile_pool(name="vp", bufs=1) as vpool,
            tc.tile_pool(name="actp", bufs=1) as actpool,
            tc.tile_pool(name="oap", bufs=1) as oa_pool,
            tc.tile_pool(name="odp", bufs=1) as od_pool,
        ):
            # PSUM pools are phase-scoped: mm1+mm2 use 2+4 banks, released
            # before mm3 opens a 6-deep eviction pipeline.
            ps_s_pool = tc.alloc_tile_pool(name="pss", bufs=4, space="PSUM")
            ps_xu_pool = tc.alloc_tile_pool(name="psxu", bufs=4, space="PSUM")
            # ---- input loads, all on the sync ring in need-order ----
            # Order: x0,U0,U1,x1 (mm2 head) -> ctx,W (mm1, consumed mid-mm2)
            # -> rest of x/U -> Vt.
            if not zero_bias:
                b_sb = cpool.tile([P, UM], F32, name="b_sb")
                nc.scalar.dma_start(b_sb[:], bias_d[:])

            ctx_sb = ctxpool.tile([P, KC1, BS], F16, name="ctx_sb")
            w_sb = wpool.tile([P, KC1, RANK], F16, name="w_sb")

            # ctx/W stream first in 3-kc chunks (mm1 consumes them kc-outer
            # right after the PE warm-up), then x/U in joint need-order, then
            # Vt for mm3.
            for lo, hi in ((0, 2), (2, 5), (5, KC1)):
                nc.sync.dma_start(ctx_sb[:, lo:hi, :], ctxT_d[:, lo:hi, :])
                nc.sync.dma_start(w_sb[:, lo:hi, :], W_d[:, lo:hi, :])

            x_of_kn: dict = {}
            u_of_kn: dict = {}
            sched = [
                ("x", 0, 2), ("u", 0, 2), ("u", 2, 4), ("x", 2, 8),
                ("u", 4, 8), ("x", 8, 16), ("u", 8, 12), ("u", 12, 16),
                ("x", 16, 24), ("u", 16, 20), ("u", 20, 24),
                ("x", 24, 32), ("u", 24, 28), ("u", 28, 32),
            ]
            for kind, lo, hi in sched:
                if kind == "x":
                    t = xpool.tile([P, hi - lo, BS], F16, name=f"x{lo}")
                    nc.sync.dma_start(t[:], xT_d[:, lo:hi, :])
                    for kn in range(lo, hi):
                        x_of_kn[kn] = t[:, kn - lo, :]
                else:
                    t = upool.tile([P, hi - lo, RANK], F16, name=f"u{lo}")
                    nc.sync.dma_start(t[:], U_d[:, lo:hi, :])
                    for kn in range(lo, hi):
                        u_of_kn[kn] = t[:, kn - lo, :]

            v_sb = []
            for c in range(UM // V_CHUNK):
                vt = vpool.tile([P, V_CHUNK, RM, P], F16, name=f"v{c}")
                nc.sync.dma_start(vt[:], V3_d[:, c * V_CHUNK : (c + 1) * V_CHUNK, :, :])
                v_sb.append(vt)

            # ---- engine warm-up during the DMA fill ----
            # warm_src is POOL-memset; PE runs ~3.4us of dummy matmuls so HAM
            # un-throttles before mm1; ACT loads its Relu table.
            warm_src = cpool.tile([P, BS + P], F16, name="warm_src")
            nc.gpsimd.memset(warm_src[:], 0.0)
            act_scr = cpool.tile([P, 1], F16, name="act_scr")
            ps_warm = ps_s_pool.tile([P, BS], F32, name="ps_warm", tag="s")
            for _ in range(N_WARM_MM):
                nc.tensor.matmul(
                    ps_warm[:], lhsT=warm_src[:, BS:], rhs=warm_src[:, :BS],
                    start=True, stop=True,
                )
            dve_scr = cpool.tile([P, RM], F32, name="dve_scr")
            if not zero_bias:
                # pre-touch bias on DVE and ACT so evictions keep <=1 wait
                dve_scr2 = cpool.tile([P, UM], F32, name="dve_scr2")
                nc.vector.tensor_copy(dve_scr2[:], b_sb[:])
                act_scr2 = cpool.tile([P, UM], F32, name="act_scr2")
                nc.scalar.copy(act_scr2[:], b_sb[:])

            # ---- mm1 (kc-outer, 4 banks): runs right after PE warm-up,
            # paced by the ctx/W chunks at the stream head ----
            sT = [actpool.tile([P, BS], F32, name=f"sT{rm}") for rm in range(RM)]
            ps_s = [
                ps_s_pool.tile([P, BS], F32, name=f"ps_s{rm}", tag="s")
                for rm in range(RM)
            ]
            for kc in range(KC1):
                for rm in range(RM):
                    nc.tensor.matmul(
                        ps_s[rm][:],
                        lhsT=w_sb[:, kc, rm * P : (rm + 1) * P],
                        rhs=ctx_sb[:, kc, :],
                        start=(kc == 0),
                        stop=(kc == KC1 - 1),
                    )
            for rm in range(RM):
                nc.vector.tensor_copy(sT[rm][:], ps_s[rm][:])
            # ACT Relu table warm-up here so its table DMA doesn't clog the
            # input stream head.
            nc.scalar.activation(
                act_scr[:], warm_src[:, :1],
                mybir.ActivationFunctionType.Relu, bias=0.0,
            )

            # ---- mm2: xuT = U^T @ xT ----
            ps_xu = [
                ps_xu_pool.tile([P, BS], F32, name=f"ps_xu{rm}", tag="xu")
                for rm in range(RM)
            ]
            for kn in range(KN):
                ut = u_of_kn[kn]
                xt = x_of_kn[kn]
                for rm in range(RM):
                    nc.tensor.matmul(
                        ps_xu[rm][:],
                        lhsT=ut[:, rm * P : (rm + 1) * P],
                        rhs=xt,
                        start=(kn == 0),
                        stop=(kn == KN - 1),
                    )
            tT = [actpool.tile([P, BS], F16, name=f"tT{rm}") for rm in range(RM)]
            # DVE fence: observe sT3's completion tick on DVE so the tT
            # multiplies need only their PE wait (walrus encodes at most one
            # sync wait on DVE tensor ops).
            nc.vector.tensor_copy(dve_scr[:, :1], sT[RM - 1][:, :1])
            for rm in range(RM):
                nc.vector.tensor_mul(tT[rm][:], ps_xu[rm][:], sT[rm][:])

            ps_xu_pool.release()
            ps_s_pool.release()
            ps_o_pool = tc.alloc_tile_pool(name="pso", bufs=6, space="PSUM")

            # Phase-boundary fences: the released PSUM banks carry accessor
            # deps (PE drains, DVE tT reads) into mm3's first ops.  One
            # single-wait fence per engine absorbs them so every mm3
            # instruction keeps <=1 wait.
            # PE observes the DVE tick via a standalone ldweights (no PSUM
            # bank involved), then one dummy matmul absorbs the bank-WAW tick.
            nc.tensor.ldweights(tT[RM - 1][:, :P])
            ps_fence = ps_o_pool.tile([P, BS], F32, name="ps_fence", tag="pso")
            nc.tensor.matmul(
                ps_fence[:], lhsT=warm_src[:, BS:], rhs=warm_src[:, :BS],
                start=True, stop=True,
            )
            nc.vector.tensor_copy(dve_scr[:, 1:2], tT[RM - 1][:, :1])
            act_fence_scr = cpool.tile([P, 1], F16, name="act_fence_scr")
            nc.scalar.copy(act_fence_scr[:], tT[RM - 1][:, :1])

            # ---- mm3: outT[um] = relu(Vt[um] @ tT + bias[um]) ----
            # Evictions alternate DVE/ACT so neither engine trails the PE.
            # All stores go on the scalar (ACT) HWDGE ring, each preceded by a
            # tiny ACT "observer" copy of the source tile's last-written slice:
            # the ACT sequencer then already holds the data tick, so the store
            # itself needs only its DMA-lane wait (walrus encodes at most one
            # sync wait per DMA instruction).
            group_sizes = [8, 8, 8, 4, 2, 2]
            assert sum(group_sizes) == UM
            um0 = 0
            for g, gs in enumerate(group_sizes):
                og_d = od_pool.tile([P, gs // 2, BS], F16, name=f"ogd{g}")
                og_a = oa_pool.tile([P, gs // 2, BS], F16, name=f"oga{g}")
                obs_d = cpool.tile([P, 1], F16, name=f"obsd{g}")
                obs_a = cpool.tile([P, 1], F16, name=f"obsa{g}")
                for j in range(gs):
                    um = um0 + j
                    ps_o = ps_o_pool.tile([P, BS], F32, name="ps_o", tag="pso")
                    vt = v_sb[um // V_CHUNK][:, um % V_CHUNK, :, :]  # [P, RM, P]
                    for kr in range(RM):
                        nc.tensor.matmul(
                            ps_o[:],
                            lhsT=vt[:, kr, :],
                            rhs=tT[kr][:],
                            start=(kr == 0),
                            stop=(kr == RM - 1),
                        )
                    if zero_bias:
                        if j % 2 == 0:
                            nc.vector.tensor_scalar_max(
                                og_d[:, j // 2, :], ps_o[:], 0.0
                            )
                        else:
                            nc.scalar.activation(
                                og_a[:, j // 2, :], ps_o[:],
                                mybir.ActivationFunctionType.Relu, bias=0.0,
                            )
                    else:
                        if j % 2 == 0:
                            nc.vector.tensor_tensor(
                                og_d[:, j // 2, :], ps_o[:],
                                b_sb[:, um : um + 1].to_broadcast((P, BS)),
                                mybir.AluOpType.add,
                            )
                            nc.vector.tensor_scalar_max(
                                og_d[:, j // 2, :], og_d[:, j // 2, :], 0.0
                            )
                        else:
                            nc.scalar.activation(
                                og_a[:, j // 2, :], ps_o[:],
                                mybir.ActivationFunctionType.Relu,
                                bias=b_sb[:, um : um + 1],
                            )
                # interleaved store-back (even um tiles from og_d, odd from og_a)
                out_g = outT_d[:, um0 : um0 + gs, :].rearrange(
                    "p (o two) b -> p o two b", two=2
                )
                nc.scalar.copy(obs_d[:], og_d[:, gs // 2 - 1, :1])
                nc.scalar.dma_start(out_g[:, :, 0, :], og_d[:])
                nc.scalar.copy(obs_a[:], og_a[:, gs // 2 - 1, :1])
                nc.scalar.dma_start(out_g[:, :, 1, :], og_a[:])
                um0 += gs

            ps_o_pool.release()

    return nc


def _pack_inputs(inputs, context, U, S, V, W, bias):
    """Shard + pack the full fp32 inputs into per-core [128,...] fp16 layouts.

    S is folded into the mm1 operands: ctxT gets a 9th contraction tile that
    is a ones-row (partition 0 only), W gets a matching row carrying S, so
    sT = W_aug^T @ ctxT_aug = S + W^T @ ctxT exactly.
    """
    zero_bias = not bias.any()
    x16 = inputs.astype(np.float16)
    c16 = context.astype(np.float16)
    U_pk = np.ascontiguousarray(U.astype(np.float16).reshape(KN, P, RANK).transpose(1, 0, 2))
    W_pk = np.zeros((P, KC + 1, RANK), dtype=np.float16)
    W_pk[:, :KC, :] = W.astype(np.float16).reshape(KC, P, RANK).transpose(1, 0, 2)
    W_pk[0, KC, :] = S.astype(np.float16)
    # V3[p, um, kr, c] = V[um*128 + c, kr*128 + p]
    V3_pk = np.ascontiguousarray(
        V.astype(np.float16).reshape(UM, P, RM, P).transpose(3, 0, 2, 1)
    )
    b_pk = np.ascontiguousarray(bias.astype(np.float32).reshape(UM, P).T)

    in_maps = []
    for c in range(M):
        xs = x16[c * BS : (c + 1) * BS]  # [BS, N]
        cs = c16[c * BS : (c + 1) * BS]  # [BS, C]
        xT = np.ascontiguousarray(xs.T.reshape(KN, P, BS).transpose(1, 0, 2))
        ctxT = np.zeros((P, KC + 1, BS), dtype=np.float16)
        ctxT[:, :KC, :] = cs.T.reshape(KC, P, BS).transpose(1, 0, 2)
        ctxT[0, KC, :] = 1.0
        im = {"xT": xT, "ctxT": ctxT, "U": U_pk, "W": W_pk, "V3": V3_pk}
        if not zero_bias:
            im["bias"] = b_pk
        in_maps.append(im)
    return in_maps


_PROGRAM_CACHE = {}


def _get_program(zero_bias: bool) -> bass.Bass:
    if zero_bias not in _PROGRAM_CACHE:
        _PROGRAM_CACHE[zero_bias] = build_program(zero_bias=zero_bias)
    return _PROGRAM_CACHE[zero_bias]


def _unpack_outputs(results) -> np.ndarray:
    shards = []
    for r in results:
        outT = r["outT"]  # [P, UM, BS] fp16
        shards.append(outT.transpose(1, 0, 2).reshape(UNITS, BS).T)
    return np.concatenate(shards, axis=0).astype(np.float32)


def kernel(inputs, context, U, S, V, W, bias, _trace=False):
    bias = np.asarray(bias)
    in_maps = _pack_inputs(
        np.asarray(inputs), np.asarray(context), np.asarray(U),
        np.asarray(S), np.asarray(V), np.asarray(W), bias,
    )
    nc = _get_program(zero_bias=not bias.any())
    res = run_bass_kernel_spmd(nc, in_maps, core_ids=list(range(M)), trace=_trace)
    out = _unpack_outputs(res.results)
    if _trace:
        return out, res
    return out


# revision 31
# speedup vs baseline: 1.0204x; 1.0204x over previous
"""Trainium2 Bass kernel for nn_CADenseAdd (context-adaptive low-rank dense + ReLU).

Reference math (per batch row b):
    s_b   = S + context_b @ W                  # [RANK]
    out_b = relu((x_b @ U) * s_b @ V.T + bias) # [UNITS]

Sharding: data-parallel over batch B=2048 across 8 cores (256 rows/core);
U/S/V/W replicated.  All matmuls are done "transposed" so the contraction
dim always lands on SBUF partitions with zero on-device transposes:

    sT  = W_aug^T @ ctxT_aug      [RANK,  BS]  (S folded in on the host)
    xuT = U^T @ xT                [RANK,  BS]
    tT  = xuT * sT  (cast fp16)   [RANK,  BS]
    outT[um] = Vt[um] @ tT        [UNITS, BS]  (+bias, ReLU on eviction)

The host packs every operand into [128, ...] partition-major contiguous
layout (part of input sharding) and casts matmul operands to fp16; PSUM
accumulation is fp32, the elementwise xu*s is fp32, so the end-to-end
relative error stays ~6e-4 vs the fp32 reference.

Pipeline (measured ~56-58 us/core on HW): all input loads stream on the
sync HWDGE ring in need-order (ctx/W -> x/U interleaved -> Vt); the PE
warms up on dummy matmuls during the DMA fill, runs mm1 as soon as the
first ctx/W chunks land, then mm2 jointly paced with the x/U stream,
then mm3 PE-dense while Vt streams in; evictions alternate DVE/ACT and
stores go out on the scalar ring.
"""

import re

import numpy as np

import bass_rust
import concourse.bass as bass
import concourse.tile as tile
from concourse import mybir
from concourse.bass_utils import run_bass_kernel_spmd
from concourse.vector_clock import ScopedClock


def _split_drain_and_barrier(self, tick_clock, wait_clock):
    """Replacement for TileContext._drain_and_barrier.

    The walrus build in this toolchain cannot encode more than one sync
    wait per instruction ("Too many sync wait commands"), and Tile's final
    drain carries one wait per active proc (~12 here).  Emit those waits as
    a chain of single-wait SP nops instead, then a bare drain: the SP queue
    executes in order, so the drain still happens after every proc's final
    tick.
    """
    ticks = [int(x) for x in re.findall(r"\d+", repr(tick_clock.global_clock))]
    for proc, tick in enumerate(ticks):
        if tick > 0:
            nop_inst = self.nc.sync.nop(nofuse=True)
            sub = bass_rust.VectorClock()
            sub.require_at_least(proc, tick)
            wait_clock.add_sem_waits(nop_inst.ins, ScopedClock({None: sub}))
    self.nc.sync.drain()
    self.nc.all_engine_barrier()
    popped = self.nc._tile_sem_poison_stack.pop()
    assert popped is self._sem_poison
    self.nc.clear_and_free_semaphores(list(self.sems.allocated().values()))
    self.nc.all_engine_barrier()


tile.TileContext._drain_and_barrier = _split_drain_and_barrier

# Problem shape (hardcoded per contract)
M = 8  # cores
B, N, C = 2048, 4096, 1024
UNITS, RANK = 4096, 512
BS = B // M  # 256 rows per core
P = 128
KN = N // P      # 32 contraction tiles for x @ U
KC = C // P      # 8 contraction tiles for ctx @ W
RM = RANK // P   # 4 tiles of RANK
UM = UNITS // P  # 32 tiles of UNITS

F16 = mybir.dt.float16
F32 = mybir.dt.float32

V_CHUNK = 4   # um tiles per Vt DMA (8 DMAs x 512 KB)


N_WARM_MM = 16  # ~3.4us of cold-rate matmuls: spans the HAM SHORT window


def build_program(zero_bias: bool = True) -> bass.Bass:
    """Build the per-core SPMD program.

    Wait-encoding constraint: this walrus build cannot encode >1 sem-wait
    on DVE/ACT tensor instructions (setupSyncWait "Too many sync wait
    commands"), while matmuls were observed to encode 2.  The structure
    below keeps every DVE/ACT instruction at <=1 wait: each engine
    "pre-touches" its DMA-sourced operands once (so later instructions
    only wait on PE), PSUM banks are never shared across phases
    (2+4+2 = 8 banks), and output staging tiles are never reused.
    """
    nc = bass.Bass("TRN2", debug=False)

    # S is folded into mm1 on the host: ctxT/W carry an extra contraction
    # tile (ones-row / S-row), so sT = W_aug^T @ ctxT_aug exactly.
    KC1 = KC + 1
    xT_d = nc.dram_tensor("xT", [P, KN, BS], F16, kind="ExternalInput").ap()
    ctxT_d = nc.dram_tensor("ctxT", [P, KC1, BS], F16, kind="ExternalInput").ap()
    U_d = nc.dram_tensor("U", [P, KN, RANK], F16, kind="ExternalInput").ap()
    W_d = nc.dram_tensor("W", [P, KC1, RANK], F16, kind="ExternalInput").ap()
    V3_d = nc.dram_tensor("V3", [P, UM, RM, P], F16, kind="ExternalInput").ap()
    if not zero_bias:
        bias_d = nc.dram_tensor("bias", [P, UM], F32, kind="ExternalInput").ap()
    outT_d = nc.dram_tensor("outT", [P, UM, BS], F16, kind="ExternalOutput").ap()

    with tile.TileContext(nc) as tc:
        with (
            tc.tile_pool(name="consts", bufs=1) as cpool,
            tc.tile_pool(name="ctxp", bufs=1) as ctxpool,
            tc.tile_pool(name="wp", bufs=1) as wpool,
            tc.tile_pool(name="xp", bufs=1) as xpool,
            tc.tile_pool(name="up", bufs=1) as upool,
            tc.tile_pool(name="vp", bufs=1) as vpool,
            tc.tile_pool(name="actp", bufs=1) as actpool,
            tc.tile_pool(name="oap", bufs=1) as oa_pool,
            tc.tile_pool(name="odp", bufs=1) as od_pool,
        ):
            # PSUM pools are phase-scoped: mm1+mm2 use 2+4 banks, released
            # before mm3 opens a 6-deep eviction pipeline.
            ps_s_pool = tc.alloc_tile_pool(name="pss", bufs=4, space="PSUM")
            ps_xu_pool = tc.alloc_tile_pool(name="psxu", bufs=4, space="PSUM")
            # ---- input loads, all on the sync ring in need-order ----
            # Order: x0,U0,U1,x1 (mm2 head) -> ctx,W (mm1, consumed mid-mm2)
            # -> rest of x/U -> Vt.
            if not zero_bias:
                b_sb = cpool.tile([P, UM], F32, name="b_sb")
                nc.scalar.dma_start(b_sb[:], bias_d[:])

            ctx_sb = ctxpool.tile([P, KC1, BS], F16, name="ctx_sb")
            w_sb = wpool.tile([P, KC1, RANK], F16, name="w_sb")

            # ctx/W stream first in 3-kc chunks (mm1 consumes them kc-outer
            # right after the PE warm-up), then x/U in joint need-order, then
            # Vt for mm3.
            for lo, hi in ((0, 2), (2, 5), (5, KC1)):
                nc.sync.dma_start(ctx_sb[:, lo:hi, :], ctxT_d[:, lo:hi, :])
                nc.sync.dma_start(w_sb[:, lo:hi, :], W_d[:, lo:hi, :])

            x_of_kn: dict = {}
            u_of_kn: dict = {}
            sched = [
                ("x", 0, 2), ("u", 0, 2), ("u", 2, 4), ("x", 2, 8),
                ("u", 4, 8), ("x", 8, 16), ("u", 8, 12), ("u", 12, 16),
                ("x", 16, 24), ("u", 16, 20), ("u", 20, 24),
                ("x", 24, 32), ("u", 24, 28), ("u", 28, 32),
            ]
            for kind, lo, hi in sched:
                if kind == "x":
                    t = xpool.tile([P, hi - lo, BS], F16, name=f"x{lo}")
                    nc.sync.dma_start(t[:], xT_d[:, lo:hi, :])
                    for kn in range(lo, hi):
                        x_of_kn[kn] = t[:, kn - lo, :]
                else:
                    t = upool.tile([P, hi - lo, RANK], F16, name=f"u{lo}")
                    nc.sync.dma_start(t[:], U_d[:, lo:hi, :])
                    for kn in range(lo, hi):
                        u_of_kn[kn] = t[:, kn - lo, :]

            v_sb = []
            for c in range(UM // V_CHUNK):
                vt = vpool.tile([P, V_CHUNK, RM, P], F16, name=f"v{c}")
                nc.sync.dma_start(vt[:], V3_d[:, c * V_CHUNK : (c + 1) * V_CHUNK, :, :])
                v_sb.append(vt)

            # ---- engine warm-up during the DMA fill ----
            # warm_src is POOL-memset; PE runs ~3.4us of dummy matmuls so HAM
            # un-throttles before mm1; ACT loads its Relu table.
            warm_src = cpool.tile([P, BS + P], F16, name="warm_src")
            nc.gpsimd.memset(warm_src[:], 0.0)
            act_scr = cpool.tile([P, 1], F16, name="act_scr")
            ps_warm = ps_s_pool.tile([P, BS], F32, name="ps_warm", tag="s")
            for _ in range(N_WARM_MM):
                nc.tensor.matmul(
                    ps_warm[:], lhsT=warm_src[:, BS:], rhs=warm_src[:, :BS],
                    start=True, stop=True,
                )
            dve_scr = cpool.tile([P, RM], F32, name="dve_scr")
            if not zero_bias:
                # pre-touch bias on DVE and ACT so evictions keep <=1 wait
                dve_scr2 = cpool.tile([P, UM], F32, name="dve_scr2")
                nc.vector.tensor_copy(dve_scr2[:], b_sb[:])
                act_scr2 = cpool.tile([P, UM], F32, name="act_scr2")
                nc.scalar.copy(act_scr2[:], b_sb[:])

            # ---- mm1 (kc-outer, 4 banks): runs right after PE warm-up,
            # paced by the ctx/W chunks at the stream head ----
            sT = [actpool.tile([P, BS], F32, name=f"sT{rm}") for rm in range(RM)]
            ps_s = [
                ps_s_pool.tile([P, BS], F32, name=f"ps_s{rm}", tag="s")
                for rm in range(RM)
            ]
            for kc in range(KC1):
                for rm in range(RM):
                    nc.tensor.matmul(
                        ps_s[rm][:],
                        lhsT=w_sb[:, kc, rm * P : (rm + 1) * P],
                        rhs=ctx_sb[:, kc, :],
                        start=(kc == 0),
                        stop=(kc == KC1 - 1),
                    )
            for rm in range(RM):
                nc.vector.tensor_copy(sT[rm][:], ps_s[rm][:])
            # ACT Relu table warm-up here so its table DMA doesn't clog the
            # input stream head.
            nc.scalar.activation(
                act_scr[:], warm_src[:, :1],
                mybir.ActivationFunctionType.Relu, bias=0.0,
            )

            # ---- mm2: xuT = U^T @ xT ----
            ps_xu = [
                ps_xu_pool.tile([P, BS], F32, name=f"ps_xu{rm}", tag="xu")
                for rm in range(RM)
            ]
            for kn in range(KN):
                ut = u_of_kn[kn]
                xt = x_of_kn[kn]
                for rm in range(RM):
                    nc.tensor.matmul(
                        ps_xu[rm][:],
                        lhsT=ut[:, rm * P : (rm + 1) * P],
                        rhs=xt,
                        start=(kn == 0),
                        stop=(kn == KN - 1),
                    )
            tT = [actpool.tile([P, BS], F16, name=f"tT{rm}") for rm in range(RM)]
            # DVE fence: observe sT3's completion tick on DVE so the tT
            # multiplies need only their PE wait (walrus encodes at most one
            # sync wait on DVE tensor ops).
            nc.vector.tensor_copy(dve_scr[:, :1], sT[RM - 1][:, :1])
            for rm in range(RM):
                nc.vector.tensor_mul(tT[rm][:], ps_xu[rm][:], sT[rm][:])

            ps_xu_pool.release()
            ps_s_pool.release()
            ps_o_pool = tc.alloc_tile_pool(name="pso", bufs=6, space="PSUM")

            # Phase-boundary fences: the released PSUM banks carry accessor
            # deps (PE drains, DVE tT reads) into mm3's first ops.  One
            # single-wait fence per engine absorbs them so every mm3
            # instruction keeps <=1 wait.
            # PE observes the DVE tick via a standalone ldweights (no PSUM
            # bank involved), then one dummy matmul absorbs the bank-WAW tick.
            nc.tensor.ldweights(tT[RM - 1][:, :P])
            ps_fence = ps_o_pool.tile([P, BS], F32, name="ps_fence", tag="pso")
            nc.tensor.matmul(
                ps_fence[:], lhsT=warm_src[:, BS:], rhs=warm_src[:, :BS],
                start=True, stop=True,
            )
            nc.vector.tensor_copy(dve_scr[:, 1:2], tT[RM - 1][:, :1])
            act_fence_scr = cpool.tile([P, 1], F16, name="act_fence_scr")
            nc.scalar.copy(act_fence_scr[:], tT[RM - 1][:, :1])

            # ---- mm3: outT[um] = relu(Vt[um] @ tT + bias[um]) ----
            # Evictions alternate DVE/ACT so neither engine trails the PE.
            # All stores go on the scalar (ACT) HWDGE ring, each preceded by a
            # tiny ACT "observer" copy of the source tile's last-written slice:
            # the ACT sequencer then already holds the data tick, so the store
            # itself needs only its DMA-lane wait (walrus encodes at most one
            # sync wait per DMA instruction).
            group_sizes = [8, 8, 8, 4, 2, 2]
            assert sum(group_sizes) == UM
            um0 = 0
            for g, gs in enumerate(group_sizes):
                og_d = od_pool.tile([P, gs // 2, BS], F16, name=f"ogd{g}")
                og_a = oa_pool.tile([P, gs // 2, BS], F16, name=f"oga{g}")
                obs_d = cpool.tile([P, 1], F16, name=f"obsd{g}")
                obs_a = cpool.tile([P, 1], F16, name=f"obsa{g}")
                for j in range(gs):
                    um = um0 + j
                    ps_o = ps_o_pool.tile([P, BS], F32, name="ps_o", tag="pso")
                    vt = v_sb[um // V_CHUNK][:, um % V_CHUNK, :, :]  # [P, RM, P]
                    for kr in range(RM):
                        nc.tensor.matmul(
                            ps_o[:],
                            lhsT=vt[:, kr, :],
                            rhs=tT[kr][:],
                            start=(kr == 0),
                            stop=(kr == RM - 1),
                        )
                    if zero_bias:
                        if j % 2 == 0:
                            nc.vector.tensor_scalar_max(
                                og_d[:, j // 2, :], ps_o[:], 0.0
                            )
                        else:
                            nc.scalar.activation(
                                og_a[:, j // 2, :], ps_o[:],
                                mybir.ActivationFunctionType.Relu, bias=0.0,
                            )
                    else:
                        if j % 2 == 0:
                            nc.vector.tensor_tensor(
                                og_d[:, j // 2, :], ps_o[:],
                                b_sb[:, um : um + 1].to_broadcast((P, BS)),
                                mybir.AluOpType.add,
                            )
                            nc.vector.tensor_scalar_max(
                                og_d[:, j // 2, :], og_d[:, j // 2, :], 0.0
                            )
                        else:
                            nc.scalar.activation(
                                og_a[:, j // 2, :], ps_o[:],
                                mybir.ActivationFunctionType.Relu,
                                bias=b_sb[:, um : um + 1],
                            )
                # interleaved store-back (even um tiles from og_d, odd from og_a)
                out_g = outT_d[:, um0 : um0 + gs, :].rearrange(
                    "p (o two) b -> p o two b", two=2
                )
                nc.scalar.copy(obs_d[:], og_d[:, gs // 2 - 1, :1])
                nc.scalar.dma_start(out_g[:, :, 0, :], og_d[:])
                nc.scalar.copy(obs_a[:], og_a[:, gs // 2 - 1, :1])
                nc.scalar.dma_start(out_g[:, :, 1, :], og_a[:])
                um0 += gs

            ps_o_pool.release()

    return nc


def _pack_inputs(inputs, context, U, S, V, W, bias):
    """Shard + pack the full fp32 inputs into per-core [128,...] fp16 layouts.

    S is folded into the mm1 operands: ctxT gets a 9th contraction tile that
    is a ones-row (partition 0 only), W gets a matching row carrying S, so
    sT = W_aug^T @ ctxT_aug = S + W^T @ ctxT exactly.
    """
    zero_bias = not bias.any()
    x16 = inputs.astype(np.float16)
    c16 = context.astype(np.float16)
    U_pk = np.ascontiguousarray(U.astype(np.float16).reshape(KN, P, RANK).transpose(1, 0, 2))
    W_pk = np.zeros((P, KC + 1, RANK), dtype=np.float16)
    W_pk[:, :KC, :] = W.astype(np.float16).reshape(KC, P, RANK).transpose(1, 0, 2)
    W_pk[0, KC, :] = S.astype(np.float16)
    # V3[p, um, kr, c] = V[um*128 + c, kr*128 + p]
    V3_pk = np.ascontiguousarray(
        V.astype(np.float16).reshape(UM, P, RM, P).transpose(3, 0, 2, 1)
    )
    b_pk = np.ascontiguousarray(bias.astype(np.float32).reshape(UM, P).T)

    in_maps = []
    for c in range(M):
        xs = x16[c * BS : (c + 1) * BS]  # [BS, N]
        cs = c16[c * BS : (c + 1) * BS]  # [BS, C]
        xT = np.ascontiguousarray(xs.T.reshape(KN, P, BS).transpose(1, 0, 2))
        ctxT = np.zeros((P, KC + 1, BS), dtype=np.float16)
        ctxT[:, :KC, :] = cs.T.reshape(KC, P, BS).transpose(1, 0, 2)
        ctxT[0, KC, :] = 1.0
        im = {"xT": xT, "ctxT": ctxT, "U": U_pk, "W": W_pk, "V3": V3_pk}
        if not zero_bias:
            im["bias"] = b_pk
        in_maps.append(im)
    return in_maps


_PROGRAM_CACHE = {}


def _get_program(zero_bias: bool) -> bass.Bass:
    if zero_bias not in _PROGRAM_CACHE:
        _PROGRAM_CACHE[zero_bias] = build_program(zero_bias=zero_bias)
    return _PROGRAM_CACHE[zero_bias]


def _unpack_outputs(results) -> np.ndarray:
    shards = []
    for r in results:
        outT = r["outT"]  # [P, UM, BS] fp16
        shards.append(outT.transpose(1, 0, 2).reshape(UNITS, BS).T)
    return np.concatenate(shards, axis=0).astype(np.float32)


def kernel(inputs, context, U, S, V, W, bias, _trace=False):
    bias = np.asarray(bias)
    in_maps = _pack_inputs(
        np.asarray(inputs), np.asarray(context), np.asarray(U),
        np.asarray(S), np.asarray(V), np.asarray(W), bias,
    )
    nc = _get_program(zero_bias=not bias.any())
    res = run_bass_kernel_spmd(nc, in_maps, core_ids=list(range(M)), trace=_trace)
    out = _unpack_outputs(res.results)
    if _trace:
        return out, res
    return out
